# revision 6
# baseline (speedup 1.0000x reference)
"""ConvDualAttention Trainium2 kernel (Bass/Tile), 8-core data-parallel.

Contract: kernel(**inputs) takes the FULL unsharded inputs (see shapes
below), shards batch b across the 8 NeuronCores (one batch per core),
and returns the full (8, 128, 4096) float32 output.

Math (per batch b, per head h, D=128, X=4096):
  y_p   = dwconv3(x) + t_p/s_p            (p in q,k,v; bias folded so that
                                           W_eff_p @ y_p == pw_p @ BN(conv))
  q,k,v = W_eff_p @ y_p                   (per-head 128-row slices)
  sk    = softmax(k over d)               (exp w/o max-sub; values are O(1))
  kat   = SCALE * q^T @ sk                (SCALE folded into W_q)
  gout  = GW @ q + gb ; sig = sigmoid(gout)
  out_h = v @ kat + sig^T * v
  out   = out_w @ merge(out_h) + out_b

Kernel-side factorization: v@kat through the output projection collapses to
  W3 @ y_v  with  W3 = sum_h outw_h @ (Wv_h^T @ kat_h)^T,
computed on-chip from the tiny per-head kat matrices.
"""
import numpy as np

import concourse.tile as tile
from concourse import bacc, mybir
from concourse.bass_utils import run_bass_kernel_spmd

F32 = mybir.dt.float32
F32R = mybir.dt.float32r
BF16 = mybir.dt.bfloat16
AF = mybir.ActivationFunctionType
ALU = mybir.AluOpType

B = 8
DIM = 128
HEADS = 8
INNER = DIM * HEADS
X = 4096
EPS = 1e-5
SCALE = DIM ** -0.5
NT = X // 128          # 32 x-tiles of 128
NCH = X // 512         # 8 chunks of 512
GROUPS = 2
GH = HEADS // GROUPS   # 4 heads per group

_NC = None
TRACE = False
LAST_EXEC_NS = None


def _prep(inputs):
    """Host-side weight folding. Returns dict of DRAM input arrays."""
    f = lambda k: np.asarray(inputs[k], np.float32)
    wt = {}
    tprime = {}
    diag_cols = []
    for p in ("q", "k", "v"):
        s = f(p + "_g") / np.sqrt(f(p + "_v") + EPS)        # (128,)
        t = f(p + "_b") - f(p + "_m") * s
        tprime[p] = t / s
        w_eff = f(p + "_pw") * s[None, :]                    # (1024, 128)
        wt[p] = np.ascontiguousarray(w_eff.T)                # (128, 1024)
        dw = f(p + "_dw")[:, 0, :]                           # (128, 3)
        for j in range(3):
            diag_cols.append(np.diag(dw[:, j]).astype(np.float32))
    s_gt = f("gt_g") / np.sqrt(f("gt_v") + EPS)
    t_gt = f("gt_b") - f("gt_m") * s_gt
    gw = f("gt_pw") * (f("gt_dw")[:, 0, 0] * s_gt)[None, :]  # (128, 128)
    gb = f("gt_pw") @ t_gt                                   # (128,)
    w_eff_q = wt["q"].T                                      # (1024, 128)
    gqt = np.concatenate(
        [(gw @ w_eff_q[h * 128:(h + 1) * 128, :]).T for h in range(HEADS)], axis=1
    )                                                        # (128 i, 1024 h*o)
    out_w = f("out_w")                                       # (128, 1024)
    outwt = np.concatenate(
        [np.ascontiguousarray(out_w[:, h * 128:(h + 1) * 128].T) for h in range(HEADS)],
        axis=1,
    )                                                        # (128 d, 1024 h*o)
    wvdm = np.concatenate(
        [wt["v"].T[h * 128:(h + 1) * 128, :] for h in range(HEADS)], axis=1
    )                                                        # (128 d, 1024 h*i)
    diag = np.concatenate(diag_cols, axis=1)                 # (128, 1152)
    biasp = np.stack(
        [tprime["q"], tprime["k"], tprime["v"], gb, f("out_b")], axis=1
    )                                                        # (128, 5)
    return {
        "wtq": np.ascontiguousarray(wt["q"] * SCALE),
        "wtk": np.ascontiguousarray(wt["k"]),
        "wtv": np.ascontiguousarray(wt["v"]),
        "gqt": np.ascontiguousarray(gqt),
        "outwt": np.ascontiguousarray(outwt),
        "wvdm": np.ascontiguousarray(wvdm),
        "diag": np.ascontiguousarray(diag),
        "biasp": np.ascontiguousarray(biasp),
    }


def _build():
    nc = bacc.Bacc("TRN2", target_bir_lowering=False, debug=False, num_devices=B)
    x_d = nc.dram_tensor("x", [128, X + 2], F32R, kind="ExternalInput").ap()
    wtq_d = nc.dram_tensor("wtq", [128, INNER], F32R, kind="ExternalInput").ap()
    wtk_d = nc.dram_tensor("wtk", [128, INNER], F32R, kind="ExternalInput").ap()
    wtv_d = nc.dram_tensor("wtv", [128, INNER], F32R, kind="ExternalInput").ap()
    gqt_d = nc.dram_tensor("gqt", [128, INNER], F32R, kind="ExternalInput").ap()
    outwt_d = nc.dram_tensor("outwt", [128, INNER], F32R, kind="ExternalInput").ap()
    wvdm_d = nc.dram_tensor("wvdm", [128, INNER], F32R, kind="ExternalInput").ap()
    diag_d = nc.dram_tensor("diag", [128, 9 * 128], F32R, kind="ExternalInput").ap()
    biasp_d = nc.dram_tensor("biasp", [128, 5], F32, kind="ExternalInput").ap()
    out_d = nc.dram_tensor("out", [128, X], F32, kind="ExternalOutput").ap()

    with tile.TileContext(nc) as tc:
        with (
            tc.tile_pool(name="const", bufs=1) as cp,
            tc.tile_pool(name="grp", bufs=1) as gp,
            tc.tile_pool(name="small", bufs=2) as sp,
        ):
            xpad = cp.tile([128, X + 2], F32R)
            wtq = cp.tile([128, INNER], F32R)
            wtk = cp.tile([128, INNER], F32R)
            wtv = cp.tile([128, INNER], F32R)
            gqt = cp.tile([128, INNER], F32R)
            outwt = cp.tile([128, INNER], F32R)
            wvdm = cp.tile([128, INNER], F32R)
            diag = cp.tile([128, 9 * 128], F32R)
            biasp = cp.tile([128, 5], F32)
            yq = cp.tile([128, X], F32R, tag="yq")
            yk = cp.tile([128, X], F32R, tag="yk")
            yv = cp.tile([128, X], F32R, tag="yv")
            zt = cp.tile([128, NT * GH], F32, tag="zt")
            zi = cp.tile([128, NT * GH], F32, tag="zi")
            w3t_sb = cp.tile([128, 128], F32R, tag="w3t")

            nc.sync.dma_start(out=xpad, in_=x_d)
            for sb_t, dr in ((wtq, wtq_d), (wtk, wtk_d), (wtv, wtv_d),
                             (gqt, gqt_d), (outwt, outwt_d), (wvdm, wvdm_d),
                             (diag, diag_d), (biasp, biasp_d)):
                nc.sync.dma_start(out=sb_t, in_=dr)

            ys = {"q": yq, "k": yk, "v": yv}

            # ---- y-stage: depthwise conv via 3 shifted diagonal matmuls ----
            with tc.tile_pool(name="yps", bufs=2, space="PSUM") as yps:
                for pi, p in enumerate(("q", "k", "v")):
                    for c in range(NCH):
                        pt = yps.tile([128, 512], F32, tag="yps")
                        for j in range(3):
                            dsl = diag[:, (pi * 3 + j) * 128:(pi * 3 + j + 1) * 128]
                            nc.tensor.matmul(
                                pt, dsl,
                                xpad[:, c * 512 + j:c * 512 + j + 512],
                                start=(j == 0), stop=(j == 2),
                            )
                        nc.vector.tensor_scalar(
                            ys[p][:, c * 512:(c + 1) * 512], pt,
                            biasp[:, pi:pi + 1], None, ALU.add,
                        )

            # ---- phase A: K softmax, Q, kat, M2T, W3T (per 4-head group) ----
            with (
                tc.tile_pool(name="kqps", bufs=2, space="PSUM") as kqps,
                tc.tile_pool(name="katps", bufs=2, space="PSUM") as katps,
                tc.tile_pool(name="m2ps", bufs=1, space="PSUM") as m2ps,
                tc.tile_pool(name="w3ps", bufs=1, space="PSUM") as w3ps,
            ):
                w3t_ps = w3ps.tile([128, 128], F32)
                for g in range(GROUPS):
                    osl = slice(g * 512, (g + 1) * 512)
                    qsb = gp.tile([128, NT * 512], BF16, tag="qsb")
                    sksb = gp.tile([128, NT * 512], BF16, tag="sksb")
                    # K production + exp evac (2 x-tiles per psum tile)
                    for tt in range(0, NT, 2):
                        kps = kqps.tile([128, 1024], F32, tag="kq")
                        for d in range(2):
                            t = tt + d
                            nc.tensor.matmul(
                                kps[:, d * 512:(d + 1) * 512],
                                yk[:, t * 128:(t + 1) * 128],
                                wtk[:, osl], start=True, stop=True,
                            )
                        nc.scalar.activation(
                            sksb[:, tt * 512:(tt + 2) * 512], kps, AF.Exp,
                        )
                    # Z per (x-tile, head), reciprocal, scale SK in place
                    for t in range(NT):
                        for hh in range(GH):
                            sl = sksb[:, t * 512 + hh * 128:t * 512 + (hh + 1) * 128]
                            nc.vector.tensor_scalar(
                                sl, sl, 1.0, 0.0, ALU.mult, ALU.add,
                                accum_out=zt[:, t * GH + hh:t * GH + hh + 1],
                            )
                        nc.vector.reciprocal(
                            zi[:, t * GH:(t + 1) * GH], zt[:, t * GH:(t + 1) * GH]
                        )
                        for hh in range(GH):
                            sl = sksb[:, t * 512 + hh * 128:t * 512 + (hh + 1) * 128]
                            nc.vector.tensor_scalar(
                                sl, sl, zi[:, t * GH + hh:t * GH + hh + 1],
                                None, ALU.mult,
                            )
                    # Q production + evac (bf16)
                    for tt in range(0, NT, 2):
                        qps = kqps.tile([128, 1024], F32, tag="kq")
                        for d in range(2):
                            t = tt + d
                            nc.tensor.matmul(
                                qps[:, d * 512:(d + 1) * 512],
                                yq[:, t * 128:(t + 1) * 128],
                                wtq[:, osl], start=True, stop=True,
                            )
                        nc.scalar.copy(qsb[:, tt * 512:(tt + 2) * 512], qps)
                    # kat -> M2T -> W3T per head
                    for hh in range(GH):
                        h = g * GH + hh
                        kat_ps = katps.tile([128, 128], F32, tag="katps")
                        for t in range(NT):
                            qs = qsb[:, t * 512 + hh * 128:t * 512 + (hh + 1) * 128]
                            ss = sksb[:, t * 512 + hh * 128:t * 512 + (hh + 1) * 128]
                            nc.tensor.matmul(
                                kat_ps, qs, ss, start=(t == 0), stop=(t == NT - 1)
                            )
                        kat_sb = sp.tile([128, 128], F32R, tag="katsb")
                        nc.vector.tensor_copy(kat_sb, kat_ps)
                        m2_ps = m2ps.tile([128, 128], F32, tag="m2")
                        nc.tensor.matmul(
                            m2_ps, kat_sb,
                            wvdm[:, h * 128:(h + 1) * 128],
                            start=True, stop=True, skip_group_check=True,
                        )
                        m2_sb = sp.tile([128, 128], F32R, tag="m2sb")
                        nc.vector.tensor_copy(m2_sb, m2_ps)
                        nc.tensor.matmul(
                            w3t_ps, m2_sb,
                            outwt[:, h * 128:(h + 1) * 128],
                            start=(h == 0), stop=(h == HEADS - 1),
                            skip_group_check=True,
                        )
                nc.vector.tensor_copy(w3t_sb, w3t_ps)

            # ---- phase B: gate + final projection per 512-chunk ----
            with (
                tc.tile_pool(name="goutps", bufs=1, space="PSUM") as goutps,
                tc.tile_pool(name="vps", bufs=1, space="PSUM") as vps,
                tc.tile_pool(name="finps", bufs=2, space="PSUM") as finps,
                tc.tile_pool(name="bpool", bufs=2) as bp,
            ):
                for c in range(NCH):
                    csl = slice(c * 512, (c + 1) * 512)
                    fin_ps = finps.tile([128, 512], F32, tag="fin")
                    nc.tensor.matmul(
                        fin_ps, w3t_sb, yv[:, csl],
                        start=True, stop=False, skip_group_check=True,
                    )
                    for hp in range(HEADS // 2):
                        g_ps = goutps.tile([128, 1024], F32, tag="gout")
                        for d in range(2):
                            h = hp * 2 + d
                            nc.tensor.matmul(
                                g_ps[:, d * 512:(d + 1) * 512],
                                gqt[:, h * 128:(h + 1) * 128],
                                yq[:, csl], start=True, stop=True,
                            )
                        sig = bp.tile([128, 1024], F32, tag="sig")
                        nc.scalar.activation(
                            sig, g_ps, AF.Sigmoid, bias=biasp[:, 3:4]
                        )
                        v_ps = vps.tile([128, 1024], F32, tag="vp")
                        for d in range(2):
                            h = hp * 2 + d
                            nc.tensor.matmul(
                                v_ps[:, d * 512:(d + 1) * 512],
                                wtv[:, h * 128:(h + 1) * 128],
                                yv[:, csl], start=True, stop=True,
                            )
                        gate = bp.tile([128, 1024], F32R, tag="gate")
                        nc.vector.tensor_tensor(gate, v_ps, sig, ALU.mult)
                        for d in range(2):
                            h = hp * 2 + d
                            nc.tensor.matmul(
                                fin_ps, outwt[:, h * 128:(h + 1) * 128],
                                gate[:, d * 512:(d + 1) * 512],
                                start=False, stop=(h == HEADS - 1),
                                skip_group_check=True,
                            )
                    fin_sb = bp.tile([128, 512], F32, tag="finsb")
                    nc.scalar.activation(
                        fin_sb, fin_ps, AF.Identity, bias=biasp[:, 4:5]
                    )
                    nc.sync.dma_start(out=out_d[:, csl], in_=fin_sb)

    nc.compile()
    return nc


def kernel(**inputs):
    global _NC, LAST_EXEC_NS
    host = _prep(inputs)
    if _NC is None:
        _NC = _build()
    x = np.asarray(inputs["x"], np.float32)
    in_maps = []
    for b in range(B):
        m = {"x": np.ascontiguousarray(np.pad(x[b], ((0, 0), (1, 1))))}
        m.update(host)
        in_maps.append(m)
    res = run_bass_kernel_spmd(
        _NC, in_maps, core_ids=list(range(B)), trace=TRACE
    )
    LAST_EXEC_NS = res.exec_time_ns
    return np.stack([r["out"] for r in res.results]).astype(np.float32)
